# revision 1
# baseline (speedup 1.0000x reference)
"""Pipeline T: spatial-layout depthwise (banded matmuls) + PE transpose + pointwise.

Per core (4 batches):
  x host-prepped to [b, h, ci, 114w] (W zero-padded, h-major) -> SBUF
  x_sb [112 h, (96 ci, 114 w)]   (two half-slabs of 48 ci each)
  DW:  per 4-ci group: 3 accumulated matmuls, lhsT = band B_v [112h,112i]
       (B_v[h,i] = k3[h-i+1, v]), rhs = x_sb[:, (4ci,114), cols v..v+112]
       -> PSUM [112 i, (4 ci, 112 j)]  (H-taps via band, W-taps via rhs shift)
  Q_sb [112 i, (96 ci, 112 j)]
  TR:  per j: transpose(lhsT=Q_sb[:, ci-stride gather @ j], identity)
       -> PSUM [96 ci, 112 i]; 4 j per bank; evac -> Qt_sb [96, j*112+i]
  PW:  per 4-row chunk: mt0 M=128, mt1 M=64 (even/odd chunks packed into
       one [128,448] PSUM tile via tile_position=(0,64)) -> y
"""

import numpy as np

from concourse import bacc, mybir
from concourse import tile
from concourse.bass_utils import run_bass_kernel_spmd

F32 = mybir.dt.float32
F32R = mybir.dt.float32r

B, C_IN, C_OUT, H, W = 32, 96, 192, 112, 112
N_CORES = 8
B_PER = B // N_CORES
WP = W + 2                      # 114 padded width
CI_G = 4                        # ci images per DW matmul group
N_G = C_IN // CI_G              # 24 groups per batch
RPC = 4                         # output rows per PW chunk -> N = 448
N_CHUNKS = H // RPC             # 28
CPB = 4                         # PW chunks per out-DMA block (16 rows)
N_BLOCKS = N_CHUNKS // CPB      # 7

_NC = None
LAST_RESULTS = None


def _build():
    nc = bacc.Bacc("TRN2", target_bir_lowering=False, debug=False,
                   num_devices=N_CORES)

    # x: [b, h, ci, wp]  (host pre-transposed + W-padded)
    x_d = nc.dram_tensor("x", [B_PER, H, C_IN, WP], F32R, kind="ExternalInput")
    band_d = nc.dram_tensor("band", [H, 3, H], F32R, kind="ExternalInput")
    ident_d = nc.dram_tensor("ident", [H, H], F32R, kind="ExternalInput")
    wpcT_d = nc.dram_tensor("wpcT", [C_IN, C_OUT], F32R, kind="ExternalInput")
    y_d = nc.dram_tensor("y", [B_PER, C_OUT, H, W], F32, kind="ExternalOutput")

    HALF = C_IN // 2            # 48 ci per x half-slab

    with tile.TileContext(nc) as tc:
        with (
            tc.tile_pool(name="consts", bufs=1) as consts,
            tc.tile_pool(name="xin", bufs=3) as xin,
            tc.tile_pool(name="qsb", bufs=1) as qsbp,
            tc.tile_pool(name="qtb", bufs=1) as qtbp,
            tc.tile_pool(name="ys", bufs=2) as ysp,
            tc.tile_pool(name="qp", bufs=2, space="PSUM") as qpp,
            tc.tile_pool(name="tp", bufs=2, space="PSUM") as tpp,
            tc.tile_pool(name="yp0", bufs=4, space="PSUM") as yp0p,
        ):
            band_sb = consts.tile([H, 3, H], F32R)
            nc.sync.dma_start(band_sb[:], band_d[:])
            ident_sb = consts.tile([H, H], F32R)
            nc.sync.dma_start(ident_sb[:], ident_d[:])
            wpc_sb = consts.tile([C_IN, C_OUT], F32R)
            nc.sync.dma_start(wpc_sb[:], wpcT_d[:])

            copy_ctr = 0

            for b in range(B_PER):
                # -------- load x half-slabs: [112 h, 48 ci, 114 w] --------
                xh = []
                for hf in range(2):
                    xt = xin.tile([H, HALF, WP], F32R, name=f"xh{hf}",
                                  tag="xh")
                    nc.gpsimd.dma_start(
                        xt[:], x_d[b, :, hf * HALF:(hf + 1) * HALF, :])
                    xh.append(xt)

                # -------- depthwise: banded matmuls --------
                # Q_sb [112 i, (96 ci, 112 j)]
                qsb = qsbp.tile([H, C_IN, W], F32R)
                for g3 in range((N_G + 1) // 2):
                    gs = [g for g in (2 * g3, 2 * g3 + 1) if g < N_G]
                    qps = []
                    for g in gs:
                        qp = qpp.tile([H, CI_G, W], F32, name="qp", tag="qp")
                        qps.append(qp)
                    for v in range(3):
                        for qp, g in zip(qps, gs):
                            hf, gg = divmod(g, N_G // 2)
                            rhs = xh[hf][:, gg * CI_G:(gg + 1) * CI_G,
                                         v:v + W]
                            nc.tensor.matmul(
                                qp[:], band_sb[:, v, :], rhs,
                                start=(v == 0), stop=(v == 2),
                            )
                    for qp, g in zip(qps, gs):
                        dst = qsb[:, g * CI_G:(g + 1) * CI_G, :]
                        if copy_ctr % 2 == 0:
                            nc.scalar.copy(dst, qp[:])
                        else:
                            nc.vector.tensor_copy(dst, qp[:])
                        copy_ctr += 1

                # -------- transpose: i<->ci per j column --------
                # Qt_sb [96 ci, (112 i, 112 j)]  addr = i*112 + j
                qtb = qtbp.tile([C_IN, H, W], F32R)
                for j4 in range(W // 4):
                    tp = tpp.tile([C_IN, 4, H], F32R, name="tp", tag="tp")
                    for jj in range(4):
                        j = 4 * j4 + jj
                        lhsT = qsb[:, :, j]          # [112 i, 96 ci] stride W
                        nc.tensor.transpose(
                            tp[:, jj, :], lhsT, ident_sb[:])
                    dst = qtb[:, :, 4 * j4:4 * j4 + 4]\
                        .rearrange("c i j -> c j i")
                    if copy_ctr % 2 == 0:
                        nc.scalar.copy(dst, tp[:])
                    else:
                        nc.vector.tensor_copy(dst, tp[:])
                    copy_ctr += 1

                # -------- pointwise: two 96-output halves --------
                # mt-major within each CPB-chunk block: 1 LDW per CPB MMs
                for blk in range(N_BLOCKS):
                    ys = [None, None]
                    for mt in range(2):
                        ys[mt] = ysp.tile([96, CPB, RPC, W], F32,
                                          name=f"ys{mt}", tag=f"ys{mt}")
                        yps = []
                        for slot in range(CPB):
                            i0 = (blk * CPB + slot) * RPC
                            rhs = qtb[:, i0:i0 + RPC, :]
                            yp = yp0p.tile([96, RPC, W], F32, name="yp",
                                           tag="yp")
                            nc.tensor.matmul(
                                yp[:], wpc_sb[:, mt * 96:(mt + 1) * 96],
                                rhs, start=True, stop=True)
                            yps.append(yp)
                        for slot, yp in enumerate(yps):
                            dst = ys[mt][:, slot, :, :]
                            if copy_ctr % 2 == 0:
                                nc.scalar.copy(dst, yp[:])
                            else:
                                nc.vector.tensor_copy(dst, yp[:])
                            copy_ctr += 1
                        r0 = blk * CPB * RPC
                        nc.sync.dma_start(
                            y_d[b, mt * 96:(mt + 1) * 96,
                                r0:r0 + CPB * RPC, :],
                            ys[mt][:].rearrange("p c r w -> p (c r) w"),
                        )

    nc.compile()
    return nc


def _prep_inputs(x, w_pc, w_dc):
    x = np.asarray(x, dtype=np.float32)
    k3 = np.asarray(w_dc, dtype=np.float32).reshape(3, 3)
    Wm = np.asarray(w_pc, dtype=np.float32).reshape(C_OUT, C_IN)

    # [b, h, ci, 114]: transpose + W-pad
    xp = np.zeros((B, H, C_IN, WP), dtype=np.float32)
    xp[:, :, :, 1:1 + W] = x.transpose(0, 2, 1, 3)

    # band[h, v, i] = k3[h - i + 1, v]
    band = np.zeros((H, 3, H), dtype=np.float32)
    hh, ii = np.meshgrid(np.arange(H), np.arange(H), indexing="ij")
    u = hh - ii + 1
    m = (u >= 0) & (u < 3)
    for v in range(3):
        bv = np.zeros((H, H), dtype=np.float32)
        bv[m] = k3[u[m], v]
        band[:, v, :] = bv

    ident = np.eye(H, dtype=np.float32)
    wpcT = np.ascontiguousarray(Wm.T)
    return xp, band, ident, wpcT


def kernel(x, w_pc, w_dc, _trace=False):
    global _NC, LAST_RESULTS
    if _NC is None:
        _NC = _build()

    xp, band, ident, wpcT = _prep_inputs(x, w_pc, w_dc)
    in_maps = [
        {"x": np.ascontiguousarray(xp[i * B_PER:(i + 1) * B_PER]),
         "band": band, "ident": ident, "wpcT": wpcT}
        for i in range(N_CORES)
    ]
    res = run_bass_kernel_spmd(_NC, in_maps, list(range(N_CORES)),
                               trace=_trace)
    LAST_RESULTS = res
    y = np.concatenate([res.results[i]["y"] for i in range(N_CORES)], axis=0)
    return np.asarray(y, dtype=np.float32)



# revision 2
# speedup vs baseline: 1.0311x; 1.0311x over previous
"""Pipeline v5: fused depthwise+transpose via x-stationary banded matmuls (bf16).

Per core (4 batches):
  x host-prepped to [b, h, 114w, ci] bf16 (W zero-padded, h on partitions)
  DW+TR fused: for each padded column w, stationary = x[:, w, :] [112h, 96ci];
    ONE matmul streams all valid taps: rhs = band3[:, t0:t0+k, :] where
    band3[:, t, :] = B_{v=2-t} (v-reversed band concat), writing k adjacent
    j-slots of a PSUM tile [96ci, 4slot, 128] (j = w-v). Accumulation across
    w via PSUM has_written bits; one start=True per bank. Splits into 2
    matmuls when the slot range crosses the bank boundary (168 MM/batch).
    -> q lands directly as [ci, i, j]: no separate PE transpose, no Q evac.
  PW: lhsT = wpcT half [96ci, 96co], rhs = qtb[:, i0:i0+4, :] (N=448),
    pairs into 2-bank PSUM tiles [96, 2, 512]; interleaved into the NEXT
    batch's DW stream (one pair per 4 w-iters) to keep the PE saturated.
  Evac: scalar/vector alternating casts f32->bf16; y written bf16, host
    upcasts to f32.
"""

import numpy as np
import ml_dtypes

from concourse import bacc, mybir
from concourse import tile
from concourse.bass_utils import run_bass_kernel_spmd

F32 = mybir.dt.float32
BF16 = mybir.dt.bfloat16

B, C_IN, C_OUT, H, W = 32, 96, 192, 112, 112
N_CORES = 8
B_PER = B // N_CORES
WP = W + 2                      # 114 padded width
SLOTS = 4                       # j-columns per DW psum bank
N_BLOCKS = W // SLOTS           # 28 DW blocks per batch
N_PAIRS = 28                    # PW matmul pairs per batch (7 blk x 2 mt x 2)

_NC = None
LAST_RESULTS = None


def _build():
    nc = bacc.Bacc("TRN2", target_bir_lowering=False, debug=False,
                   num_devices=N_CORES)

    x_d = nc.dram_tensor("x", [B_PER, H, WP, C_IN], BF16, kind="ExternalInput")
    band3_d = nc.dram_tensor("band3", [H, 3, H], BF16, kind="ExternalInput")
    wpcT_d = nc.dram_tensor("wpcT", [C_IN, C_OUT], BF16, kind="ExternalInput")
    y_d = nc.dram_tensor("y", [B_PER, C_OUT, H, W], BF16, kind="ExternalOutput")

    with tile.TileContext(nc) as tc:
        with (
            tc.tile_pool(name="consts", bufs=1) as consts,
            tc.tile_pool(name="xin", bufs=2) as xin,
            tc.tile_pool(name="qtb", bufs=2) as qtbp,
            tc.tile_pool(name="ys", bufs=4) as ysp,
            tc.tile_pool(name="dwp", bufs=3, space="PSUM") as dwp,
            tc.tile_pool(name="ypp", bufs=2, space="PSUM") as ypp,
        ):
            band3_sb = consts.tile([H, 3, H], BF16)
            nc.sync.dma_start(band3_sb[:], band3_d[:])
            wpc_sb = consts.tile([C_IN, C_OUT], BF16)
            nc.sync.dma_start(wpc_sb[:], wpcT_d[:])

            copy_ctr = 0

            def evac(dst, src):
                nonlocal copy_ctr
                if copy_ctr % 2 == 0:
                    nc.scalar.copy(dst, src)
                else:
                    nc.vector.tensor_copy(dst, src)
                copy_ctr += 1

            def load_x(b, xt):
                # 4 w-chunks so the first matmuls don't wait on the full slab
                for c in range(4):
                    w0 = [0, 32, 64, 96][c]
                    w1 = [32, 64, 96, WP][c]
                    nc.gpsimd.dma_start(xt[:, w0:w1, :], x_d[b, :, w0:w1, :])

            xt_cur = xin.tile([H, WP, C_IN], BF16, name="x0", tag="x")
            load_x(0, xt_cur)

            def pw_steps(bprev, qtb_prev):
                """Generator: one PW pair (2 matmuls + cast + maybe DMA)."""
                ys_cur = None
                for p in range(N_PAIRS):
                    blk, mt, h2 = p // 4, (p // 2) % 2, p % 2
                    if h2 == 0:
                        ys_cur = ysp.tile([C_IN, 16, W], BF16,
                                          name=f"ys{mt}", tag="ys")
                    yt = ypp.tile([C_IN, 2, 512], F32, name="yp", tag="yp")
                    lhsT = wpc_sb[:, mt * 96:(mt + 1) * 96]
                    for k in range(2):
                        i0 = blk * 16 + h2 * 8 + k * 4
                        nc.tensor.matmul(
                            yt[:, k, 0:448], lhsT,
                            qtb_prev[:, i0:i0 + 4, :],
                            start=True, stop=True)
                    evac(ys_cur[:, h2 * 8:(h2 + 1) * 8, :], yt[:, :, 0:448])
                    if h2 == 1:
                        nc.gpsimd.dma_start(
                            y_d[bprev, mt * 96:(mt + 1) * 96,
                                blk * 16:(blk + 1) * 16, :],
                            ys_cur[:])
                    yield

            pw_iter = None
            for b in range(B_PER):
                qtb = qtbp.tile([C_IN, H, W], BF16, name="qtb", tag="qtb")
                xt = xt_cur
                if b + 1 < B_PER:
                    xt_next = xin.tile([H, WP, C_IN], BF16,
                                       name=f"x{b + 1}", tag="x")
                    load_x(b + 1, xt_next)
                    xt_cur = xt_next

                ptiles = {}          # block -> psum tile
                for w in range(WP):
                    jlo, jhi = max(0, w - 2), min(W - 1, w)
                    lhsT = xt[:, w, :]
                    # group valid j-columns by psum block
                    groups = {}
                    for j in range(jlo, jhi + 1):
                        groups.setdefault(j // SLOTS, []).append(j)
                    for block in sorted(groups):
                        gj = groups[block]
                        s0 = gj[0] % SLOTS
                        t0 = 2 - (w - gj[0])
                        k = len(gj)
                        start = block not in ptiles
                        if start:
                            ptiles[block] = dwp.tile([C_IN, SLOTS, 128], F32,
                                                     name="dw", tag="dw")
                        stop = (gj[-1] == SLOTS * block + SLOTS - 1
                                and w - gj[-1] == 2)
                        nc.tensor.matmul(
                            ptiles[block][:, s0:s0 + k, 0:112],
                            lhsT, band3_sb[:, t0:t0 + k, :],
                            start=start, stop=stop, skip_group_check=True)
                    bd = (w - 5) // SLOTS
                    if bd >= 0 and bd in ptiles:
                        pt = ptiles.pop(bd)
                        evac(qtb[:, :, SLOTS * bd:SLOTS * (bd + 1)]
                             .rearrange("c i j -> c j i"),
                             pt[:, :, 0:112])
                    if w % 4 == 3 and pw_iter is not None:
                        next(pw_iter, None)
                for block, pt in sorted(ptiles.items()):
                    evac(qtb[:, :, SLOTS * block:SLOTS * (block + 1)]
                         .rearrange("c i j -> c j i"),
                         pt[:, :, 0:112])
                ptiles.clear()
                if pw_iter is not None:
                    for _ in pw_iter:
                        pass
                pw_iter = pw_steps(b, qtb)

            for _ in pw_iter:
                pass

    nc.compile()
    return nc


def _prep_inputs(x, w_pc, w_dc):
    x = np.asarray(x, dtype=np.float32)
    k3 = np.asarray(w_dc, dtype=np.float32).reshape(3, 3)
    Wm = np.asarray(w_pc, dtype=np.float32).reshape(C_OUT, C_IN)

    # [b, h, 114, ci]: transpose + W-pad
    xp = np.zeros((B, H, WP, C_IN), dtype=np.float32)
    xp[:, :, 1:1 + W, :] = x.transpose(0, 2, 3, 1)

    # band3[h, t, i] = k3[h - i + 1, 2 - t]  (v-reversed band concat)
    band3 = np.zeros((H, 3, H), dtype=np.float32)
    hh, ii = np.meshgrid(np.arange(H), np.arange(H), indexing="ij")
    u = hh - ii + 1
    m = (u >= 0) & (u < 3)
    for t in range(3):
        bv = np.zeros((H, H), dtype=np.float32)
        bv[m] = k3[u[m], 2 - t]
        band3[:, t, :] = bv

    wpcT = np.ascontiguousarray(Wm.T)
    bf = ml_dtypes.bfloat16
    return (xp.astype(bf), band3.astype(bf), wpcT.astype(bf))


def kernel(x, w_pc, w_dc, _trace=False):
    global _NC, LAST_RESULTS
    if _NC is None:
        _NC = _build()

    xp, band3, wpcT = _prep_inputs(x, w_pc, w_dc)
    in_maps = [
        {"x": np.ascontiguousarray(xp[i * B_PER:(i + 1) * B_PER]),
         "band3": band3, "wpcT": wpcT}
        for i in range(N_CORES)
    ]
    res = run_bass_kernel_spmd(_NC, in_maps, list(range(N_CORES)),
                               trace=_trace)
    LAST_RESULTS = res
    y = np.concatenate([res.results[i]["y"] for i in range(N_CORES)], axis=0)
    return np.asarray(y, dtype=np.float32)


# revision 9
# speedup vs baseline: 1.3975x; 1.3554x over previous
"""Pipeline v5: fused depthwise+transpose via x-stationary banded matmuls (bf16).

Per core (4 batches):
  x host-prepped to [b, h, 114w, ci] bf16 (W zero-padded, h on partitions)
  DW+TR fused: for each padded column w, stationary = x[:, w, :] [112h, 96ci];
    ONE matmul streams all valid taps: rhs = band3[:, t0:t0+k, :] where
    band3[:, t, :] = B_{v=2-t} (v-reversed band concat), writing k adjacent
    j-slots of a PSUM tile [96ci, 4slot, 128] (j = w-v). Accumulation across
    w via PSUM has_written bits; one start=True per bank. Splits into 2
    matmuls when the slot range crosses the bank boundary (168 MM/batch).
    -> q lands directly as [ci, i, j]: no separate PE transpose, no Q evac.
  PW: lhsT = wpcT half [96ci, 96co], rhs = qtb[:, i0:i0+4, :] (N=448),
    pairs into 2-bank PSUM tiles [96, 2, 512]; interleaved into the NEXT
    batch's DW stream (one pair per 4 w-iters) to keep the PE saturated.
  Evac: scalar/vector alternating casts f32->bf16; y written bf16, host
    upcasts to f32.
"""

import numpy as np
import ml_dtypes

from concourse import bacc, mybir
from concourse import tile
from concourse.bass_utils import run_bass_kernel_spmd

F32 = mybir.dt.float32
F32R = mybir.dt.float32r
BF16 = mybir.dt.bfloat16

B, C_IN, C_OUT, H, W = 32, 96, 192, 112, 112
N_CORES = 8
B_PER = B // N_CORES
WP = W + 2                      # 114 padded width
SLOTS = 4                       # j-columns per DW psum bank
N_BLOCKS = W // SLOTS           # 28 DW blocks per batch
N_PAIRS = 28                    # PW matmul pairs per batch (7 blk x 2 mt x 2)

_NC = None
LAST_RESULTS = None


def _build():
    nc = bacc.Bacc("TRN2", target_bir_lowering=False, debug=False,
                   num_devices=N_CORES)

    x_d = nc.dram_tensor("x", [B_PER, H, WP, C_IN], BF16, kind="ExternalInput")
    band3_d = nc.dram_tensor("band3", [H, 3, H], BF16, kind="ExternalInput")
    wpcT_d = nc.dram_tensor("wpcT", [C_IN, C_OUT], F32R, kind="ExternalInput")
    y_d = nc.dram_tensor("y", [B_PER, C_OUT, H, W], F32, kind="ExternalOutput")

    with tile.TileContext(nc) as tc:
        with (
            tc.tile_pool(name="consts", bufs=1) as consts,
            tc.tile_pool(name="xin", bufs=2) as xin,
            tc.tile_pool(name="qtb", bufs=2) as qtbp,
            tc.tile_pool(name="ys", bufs=4) as ysp,
            tc.tile_pool(name="dwp", bufs=3, space="PSUM") as dwp,
            tc.tile_pool(name="ypp", bufs=2, space="PSUM") as ypp,
        ):
            band3_sb = consts.tile([H, 3, H], BF16)
            nc.sync.dma_start(band3_sb[:], band3_d[:])
            wpc_sb = consts.tile([C_IN, C_OUT], F32R)
            nc.sync.dma_start(wpc_sb[:], wpcT_d[:])

            ctrs = {"q": 0, "y": 0}

            def evac(kind, dst, src):
                if ctrs[kind] % 2 == 0:
                    nc.scalar.copy(dst, src)
                else:
                    nc.vector.tensor_copy(dst, src)
                ctrs[kind] += 1

            def load_x(b, xt):
                # 4 w-chunks so the first matmuls don't wait on the full slab
                for c in range(4):
                    w0 = [0, 32, 64, 96][c]
                    w1 = [32, 64, 96, WP][c]
                    nc.gpsimd.dma_start(xt[:, w0:w1, :], x_d[b, :, w0:w1, :])

            xt_cur = xin.tile([H, WP, C_IN], BF16, name="x0", tag="x")
            load_x(0, xt_cur)

            def pw_steps(bprev, qtb_prev):
                """Generator: one PW pair (2 matmuls + cast + maybe DMA)."""
                ys_cur = None
                for p in range(N_PAIRS):
                    blk, mt, h2 = p // 4, (p // 2) % 2, p % 2
                    if h2 == 0:
                        ys_cur = ysp.tile([C_IN, 16, W], F32,
                                          name=f"ys{mt}", tag="ys")
                    yt = ypp.tile([C_IN, 2, 512], F32, name="yp", tag="yp")
                    lhsT = wpc_sb[:, mt * 96:(mt + 1) * 96]
                    for k in range(2):
                        i0 = blk * 16 + h2 * 8 + k * 4
                        nc.tensor.matmul(
                            yt[:, k, 0:448], lhsT,
                            qtb_prev[:, i0:i0 + 4, :],
                            start=True, stop=True)
                    evac("y", ys_cur[:, h2 * 8:(h2 + 1) * 8, :],
                         yt[:, :, 0:448])
                    if h2 == 1:
                        nc.gpsimd.dma_start(
                            y_d[bprev, mt * 96:(mt + 1) * 96,
                                blk * 16:(blk + 1) * 16, :],
                            ys_cur[:])
                    yield

            pw_iter = None
            for b in range(B_PER):
                qtb = qtbp.tile([C_IN, H, W], F32R, name="qtb", tag="qtb")
                xt = xt_cur
                if b + 1 < B_PER:
                    xt_next = xin.tile([H, WP, C_IN], BF16,
                                       name=f"x{b + 1}", tag="x")
                    load_x(b + 1, xt_next)
                    xt_cur = xt_next

                ptiles = {}          # block -> psum tile
                for w in range(WP):
                    jlo, jhi = max(0, w - 2), min(W - 1, w)
                    lhsT = xt[:, w, :]
                    # group valid j-columns by psum block
                    groups = {}
                    for j in range(jlo, jhi + 1):
                        groups.setdefault(j // SLOTS, []).append(j)
                    for block in sorted(groups):
                        gj = groups[block]
                        s0 = gj[0] % SLOTS
                        t0 = 2 - (w - gj[0])
                        k = len(gj)
                        start = block not in ptiles
                        if start:
                            ptiles[block] = dwp.tile([C_IN, SLOTS, 128], F32,
                                                     name="dw", tag="dw")
                        stop = (gj[-1] == SLOTS * block + SLOTS - 1
                                and w - gj[-1] == 2)
                        nc.tensor.matmul(
                            ptiles[block][:, s0:s0 + k, 0:112],
                            lhsT, band3_sb[:, t0:t0 + k, :],
                            start=start, stop=stop, skip_group_check=True)
                    bd = (w - 5) // SLOTS
                    if bd >= 0 and bd in ptiles:
                        pt = ptiles.pop(bd)
                        evac("q", qtb[:, :, SLOTS * bd:SLOTS * (bd + 1)]
                             .rearrange("c i j -> c j i"),
                             pt[:, :, 0:112])
                    if w % 4 == 3 and pw_iter is not None:
                        next(pw_iter, None)
                for block, pt in sorted(ptiles.items()):
                    evac("q", qtb[:, :, SLOTS * block:SLOTS * (block + 1)]
                         .rearrange("c i j -> c j i"),
                         pt[:, :, 0:112])
                ptiles.clear()
                if pw_iter is not None:
                    for _ in pw_iter:
                        pass
                pw_iter = pw_steps(b, qtb)

            for _ in pw_iter:
                pass

    nc.compile()
    return nc


def _prep_inputs(x, w_pc, w_dc):
    x = np.asarray(x, dtype=np.float32)
    k3 = np.asarray(w_dc, dtype=np.float32).reshape(3, 3)
    Wm = np.asarray(w_pc, dtype=np.float32).reshape(C_OUT, C_IN)

    # [b, h, 114, ci]: transpose + W-pad
    xp = np.zeros((B, H, WP, C_IN), dtype=np.float32)
    xp[:, :, 1:1 + W, :] = x.transpose(0, 2, 3, 1)

    # band3[h, t, i] = k3[h - i + 1, 2 - t]  (v-reversed band concat)
    band3 = np.zeros((H, 3, H), dtype=np.float32)
    hh, ii = np.meshgrid(np.arange(H), np.arange(H), indexing="ij")
    u = hh - ii + 1
    m = (u >= 0) & (u < 3)
    for t in range(3):
        bv = np.zeros((H, H), dtype=np.float32)
        bv[m] = k3[u[m], 2 - t]
        band3[:, t, :] = bv

    wpcT = np.ascontiguousarray(Wm.T)
    bf = ml_dtypes.bfloat16
    return (xp.astype(bf), band3.astype(bf), wpcT)


def kernel(x, w_pc, w_dc, _trace=False):
    global _NC, LAST_RESULTS
    if _NC is None:
        _NC = _build()

    xp, band3, wpcT = _prep_inputs(x, w_pc, w_dc)
    in_maps = [
        {"x": np.ascontiguousarray(xp[i * B_PER:(i + 1) * B_PER]),
         "band3": band3, "wpcT": wpcT}
        for i in range(N_CORES)
    ]
    res = run_bass_kernel_spmd(_NC, in_maps, list(range(N_CORES)),
                               trace=_trace)
    LAST_RESULTS = res
    y = np.concatenate([res.results[i]["y"] for i in range(N_CORES)], axis=0)
    return np.asarray(y, dtype=np.float32)
